# revision 7
# baseline (speedup 1.0000x reference)
"""MoE (top-2 of 8 experts, SwiGLU FFN) on 8 trn2 NeuronCores.

Strategy:
  Launch A (data-parallel router): each core computes fp32 logits for T/8
    tokens, softmax, top-2 combine weights (exact renormalization), and
    partial aux-loss sums (counts, prob column-sums, sum lse^2).
  Host: builds per-expert token index lists from the device combine weights
    (the all-to-all dispatch), pads to a common capacity C.
  Launch B (expert-parallel FFN): core e holds expert e's weights and its
    gathered tokens (activations kept token-in-free-dim / transposed so all
    three matmuls use natural weight layouts with zero on-device transposes).
    Matmuls run in float32r (TF32-like, 11-bit mantissa -> exact products,
    fp32 PSUM accumulate) at 4x the fp32 PE rate. The per-token combine
    weight is applied on-device to the mm3 output.
  Host: scatter-adds per-expert outputs back to [T, H] (the unshard step)
    and combines aux partials into the two scalar losses.
"""

import sys
import types

sys.path.insert(0, "/opt/trn_rl_repo")

import numpy as np

import concourse.bass as bass  # noqa: F401  (bass must import before bacc)
import concourse.mybir as mybir
import concourse.tile as tile
from concourse import bacc
from concourse.bass_utils import run_bass_kernel_spmd

H, E, TOPK, F = 1024, 8, 2, 4096
NCORES = 8
TB = 768  # token block for the expert kernel (hT resident per block)
P = 128

TRACE = False  # test harness flips this to collect NTFF exec times
LAST_EXEC_NS = {}

FP32 = mybir.dt.float32
FP32R = mybir.dt.float32r

_cache = {}


def _install_trace_hook():
    try:
        from trn_agent_boot.trn_boot import _ntff_profile_via_ctypes

        hook = _ntff_profile_via_ctypes("/opt/axon/libaxon_pjrt.so")
        m = types.ModuleType("antenv.axon_hooks")
        m.get_axon_ntff_profile_hook = lambda: hook
        sys.modules["antenv.axon_hooks"] = m
        return True
    except Exception:
        return False


def _round_f32r(a):
    """Round fp32 -> fp32r (11-bit mantissa) the way the HW/static_cast does."""
    v = np.ascontiguousarray(a, dtype=np.float32).view(np.uint32).astype(np.uint64)
    v = (v + 0x800) & 0xFFFFF000
    return v.astype(np.uint32).view(np.float32)


def _build_router(Td):
    """SPMD router kernel: Td tokens per core, batched over all Mt m-tiles.

    Inputs:  xT [H, Td] fp32 (tokens of this core, transposed), gw [H, E] fp32
    Outputs: comb [Td, E] fp32 (renormalized top-2 combine weights, 0 elsewhere)
             stats [1, 2*Mt*E + Mt] fp32 =
               [mask colsums per (m,e) | prob colsums per (m,e) | lse^2 sums per m]
    No max-subtraction in the softmax: |logits| <~ 10 here, exp() is safe and
    probs/lse match the max-subtracted form to fp rounding.
    """
    KT = H // P
    Mt = Td // P
    assert Mt * E <= 512  # logits for all m-tiles live in one PSUM bank
    nc = bacc.Bacc("TRN2", target_bir_lowering=False, debug=False, num_devices=NCORES)
    xT = nc.dram_tensor("xT", [H, Td], FP32, kind="ExternalInput")
    gw = nc.dram_tensor("gw", [H, E], FP32, kind="ExternalInput")
    comb_o = nc.dram_tensor("comb", [Td, E], FP32, kind="ExternalOutput")
    NST = 2 * Mt * E + Mt
    stats_o = nc.dram_tensor("stats", [1, NST], FP32, kind="ExternalOutput")

    AF = mybir.ActivationFunctionType
    ALU = mybir.AluOpType
    with tile.TileContext(nc) as tc:
        with (
            tc.tile_pool(name="const", bufs=1) as const,
            tc.tile_pool(name="xp", bufs=1) as xp,
            tc.tile_pool(name="sp", bufs=1) as sp,
            tc.tile_pool(name="pp", bufs=1, space="PSUM") as pp,
            tc.tile_pool(name="spp", bufs=1, space="PSUM") as spp,
        ):
            gsb = xp.tile([P, KT, E], FP32)
            nc.sync.dma_start(gsb[:], gw.rearrange("(k p) e -> p k e", p=P))
            xsb = xp.tile([P, KT, Td], FP32)
            for k in range(KT):
                nc.sync.dma_start(xsb[:, k, :], xT[k * P:(k + 1) * P, :])
            ones = const.tile([P, 1], FP32)
            nc.vector.memset(ones[:], 1.0)

            # logits for ALL tokens in one PSUM bank [128, Mt, E].
            # NOTE: accumulation groups must be contiguous in issue order —
            # k-outer interleaving of groups in one bank computes garbage.
            ps = pp.tile([P, Mt, E], FP32)
            for m in range(Mt):
                for k in range(KT):
                    nc.tensor.matmul(
                        ps[:, m, :],
                        xsb[:, k, m * P:(m + 1) * P],
                        gsb[:, k, :],
                        start=(k == 0),
                        stop=(k == KT - 1),
                    )

            def b3(t):  # [P, Mt] -> broadcast [P, Mt, E] access pattern
                return t[:].to_broadcast((P, Mt, E))

            ex = sp.tile([P, Mt, E], FP32, tag="ex")
            nc.scalar.activation(ex[:], ps[:], AF.Exp)
            sumexp = sp.tile([P, Mt], FP32, tag="sumexp")
            nc.vector.reduce_sum(sumexp[:], ex[:], axis=mybir.AxisListType.X)
            rs = sp.tile([P, Mt], FP32, tag="rs")
            nc.vector.reciprocal(rs[:], sumexp[:])
            probs = sp.tile([P, Mt, E], FP32, tag="probs")
            nc.vector.tensor_tensor(probs[:], ex[:], b3(rs), op=ALU.mult)

            top1 = sp.tile([P, Mt], FP32, tag="top1")
            nc.vector.reduce_max(top1[:], probs[:], axis=mybir.AxisListType.X)
            m1 = sp.tile([P, Mt, E], FP32, tag="m1")
            nc.vector.tensor_tensor(m1[:], probs[:], b3(top1), op=ALU.is_ge)
            pm = sp.tile([P, Mt, E], FP32, tag="pm")
            nc.vector.tensor_mul(pm[:], probs[:], m1[:])
            nc.vector.tensor_sub(pm[:], probs[:], pm[:])
            top2 = sp.tile([P, Mt], FP32, tag="top2")
            nc.vector.reduce_max(top2[:], pm[:], axis=mybir.AxisListType.X)
            mask = sp.tile([P, Mt, E], FP32, tag="mask")
            nc.vector.tensor_tensor(mask[:], pm[:], b3(top2), op=ALU.is_ge)
            nc.vector.tensor_add(mask[:], mask[:], m1[:])
            cu = sp.tile([P, Mt, E], FP32, tag="cu")
            nc.vector.tensor_mul(cu[:], probs[:], mask[:])
            denom = sp.tile([P, Mt], FP32, tag="denom")
            nc.vector.tensor_add(denom[:], top1[:], top2[:])
            rd = sp.tile([P, Mt], FP32, tag="rd")
            nc.vector.reciprocal(rd[:], denom[:])
            comb = sp.tile([P, Mt, E], FP32, tag="comb")
            nc.vector.tensor_tensor(comb[:], cu[:], b3(rd), op=ALU.mult)
            nc.sync.dma_start(comb_o.rearrange("(m p) e -> p m e", p=P), comb[:])

            lse = sp.tile([P, Mt], FP32, tag="lse")
            nc.scalar.activation(lse[:], sumexp[:], AF.Ln)
            z2 = sp.tile([P, Mt], FP32, tag="z2")
            nc.vector.tensor_mul(z2[:], lse[:], lse[:])

            # column-sums over the 128 token partitions via ones-matmul
            stats_ps = spp.tile([1, NST], FP32)
            nc.tensor.matmul(stats_ps[:, 0:Mt * E], ones[:], mask[:])
            nc.tensor.matmul(stats_ps[:, Mt * E:2 * Mt * E], ones[:], probs[:])
            nc.tensor.matmul(stats_ps[:, 2 * Mt * E:NST], ones[:], z2[:])
            stats_sb = sp.tile([1, NST], FP32, tag="stats_sb")
            nc.vector.tensor_copy(stats_sb[:], stats_ps[:])
            nc.sync.dma_start(stats_o[:], stats_sb[:])
    nc.compile()
    return nc, Mt


def _build_expert(C):
    """SPMD expert kernel: capacity C tokens per core (C multiple of TB).

    Inputs (all f32r except cv):
      xg [H, C]   gathered tokens, transposed
      wg, wu [H, F], wd [F, H]   expert weights, natural layouts
      cv [1, C] fp32             per-token combine weight (0 for padding)
    Output: yT [H, C] fp32 = comb * (silu(xg.T@wg) * (xg.T@wu)) @ wd, transposed
    """
    KT = H // P          # 8  k-tiles over H
    FT = F // P          # 32 f-tiles over F
    MT = H // P          # 8  m-tiles over H (mm3 out)
    NT = C // TB
    chunks = []
    c0 = 0
    while c0 < TB:
        cn = min(512, TB - c0)
        chunks.append((c0, cn))
        c0 += cn
    AF = mybir.ActivationFunctionType

    nc = bacc.Bacc("TRN2", target_bir_lowering=False, debug=False, num_devices=NCORES)
    xg = nc.dram_tensor("xg", [H, C], FP32R, kind="ExternalInput")
    wgd = nc.dram_tensor("wg", [H, F], FP32R, kind="ExternalInput")
    wud = nc.dram_tensor("wu", [H, F], FP32R, kind="ExternalInput")
    wdd = nc.dram_tensor("wd", [F, H], FP32R, kind="ExternalInput")
    cv = nc.dram_tensor("cv", [1, C], FP32, kind="ExternalInput")
    yT = nc.dram_tensor("yT", [H, C], FP32, kind="ExternalOutput")

    with tile.TileContext(nc) as tc:
        with (
            tc.tile_pool(name="cpool", bufs=1) as cpool,
            tc.tile_pool(name="xpool", bufs=1) as xpool,
            tc.tile_pool(name="hpool", bufs=1) as hpool,
            tc.tile_pool(name="wpool", bufs=3) as wpool,
            tc.tile_pool(name="wdpool", bufs=2) as wdpool,
            tc.tile_pool(name="h1pool", bufs=3) as h1pool,
            tc.tile_pool(name="ypool", bufs=3) as ypool,
            tc.tile_pool(name="ppa", bufs=2, space="PSUM") as ppa,
            tc.tile_pool(name="ppb", bufs=3, space="PSUM") as ppb,
        ):
            comb_sb = cpool.tile([P, C], FP32)
            nc.sync.dma_start(comb_sb[:], cv[0:1, :].to_broadcast((P, C)))

            for tb in range(NT):
                tok0 = tb * TB
                xsb = xpool.tile([P, KT, TB], FP32R, tag="xsb")
                for k in range(KT):
                    for (c0, cn) in chunks:
                        nc.sync.dma_start(
                            xsb[:, k, c0:c0 + cn],
                            xg[k * P:(k + 1) * P, tok0 + c0:tok0 + c0 + cn],
                        )
                hT = hpool.tile([P, FT, TB], FP32R, tag="hT")

                # ---- phase A: hT = f32r(silu(x@wg) * (x@wu)), F on partitions
                for f in range(FT):
                    wgsb = wpool.tile([P, KT, P], FP32R, tag="wgsb")
                    nc.sync.dma_start(
                        wgsb[:],
                        wgd[:, f * P:(f + 1) * P].rearrange("(k p) m -> p k m", p=P),
                    )
                    wusb = wpool.tile([P, KT, P], FP32R, tag="wusb")
                    nc.sync.dma_start(
                        wusb[:],
                        wud[:, f * P:(f + 1) * P].rearrange("(k p) m -> p k m", p=P),
                    )
                    for (c0, cn) in chunks:
                        ps1 = ppa.tile([P, 512], FP32, tag="ps1")
                        for k in range(KT):
                            nc.tensor.matmul(
                                ps1[:, :cn],
                                wgsb[:, k, :],
                                xsb[:, k, c0:c0 + cn],
                                start=(k == 0),
                                stop=(k == KT - 1),
                            )
                        h1 = h1pool.tile([P, 512], FP32, tag="h1")
                        nc.scalar.activation(h1[:, :cn], ps1[:, :cn], AF.Silu)
                        ps2 = ppa.tile([P, 512], FP32, tag="ps2")
                        for k in range(KT):
                            nc.tensor.matmul(
                                ps2[:, :cn],
                                wusb[:, k, :],
                                xsb[:, k, c0:c0 + cn],
                                start=(k == 0),
                                stop=(k == KT - 1),
                            )
                        nc.vector.tensor_mul(
                            hT[:, f, c0:c0 + cn], h1[:, :cn], ps2[:, :cn]
                        )

                # ---- phase B: yT = comb * (hT.T @ wd).T, H on partitions
                for m in range(MT):
                    wdsb = wdpool.tile([P, FT, P], FP32R, tag="wdsb")
                    nc.sync.dma_start(
                        wdsb[:],
                        wdd[:, m * P:(m + 1) * P].rearrange("(k p) h -> p k h", p=P),
                    )
                    for (c0, cn) in chunks:
                        ps = ppb.tile([P, 512], FP32, tag="ps3")
                        for k in range(FT):
                            nc.tensor.matmul(
                                ps[:, :cn],
                                wdsb[:, k, :],
                                hT[:, k, c0:c0 + cn],
                                start=(k == 0),
                                stop=(k == FT - 1),
                            )
                        yo = ypool.tile([P, 512], FP32, tag="yo")
                        nc.vector.tensor_mul(
                            yo[:, :cn], ps[:, :cn],
                            comb_sb[:, tok0 + c0:tok0 + c0 + cn],
                        )
                        nc.sync.dma_start(
                            yT[m * P:(m + 1) * P, tok0 + c0:tok0 + c0 + cn],
                            yo[:, :cn],
                        )
    nc.compile()
    return nc


def kernel(x, gate_w, w_gate, w_up, w_down):
    global LAST_EXEC_NS
    LAST_EXEC_NS = {}
    trace = TRACE and _install_trace_hook()

    x = np.ascontiguousarray(np.asarray(x, dtype=np.float32))
    gate_w = np.ascontiguousarray(np.asarray(gate_w, dtype=np.float32))
    w_gate = np.ascontiguousarray(np.asarray(w_gate, dtype=np.float32))
    w_up = np.ascontiguousarray(np.asarray(w_up, dtype=np.float32))
    w_down = np.ascontiguousarray(np.asarray(w_down, dtype=np.float32))

    B, S, _ = x.shape
    T = B * S
    assert T % NCORES == 0
    Td = T // NCORES
    x_flat = x.reshape(T, H)

    # ---- launch A: router
    key = ("router", Td)
    if key not in _cache:
        _cache[key] = _build_router(Td)
    ncA, Mt = _cache[key]
    in_maps = [
        {"xT": np.ascontiguousarray(x_flat[d * Td:(d + 1) * Td].T), "gw": gate_w}
        for d in range(NCORES)
    ]
    resA = run_bass_kernel_spmd(ncA, in_maps, list(range(NCORES)), trace=trace)
    if trace:
        LAST_EXEC_NS["router"] = resA.exec_time_ns
    comb = np.concatenate([r["comb"] for r in resA.results], axis=0)  # [T, E]
    stats = np.sum([r["stats"][0] for r in resA.results], axis=0)
    counts = stats[0:Mt * E].reshape(Mt, E).sum(axis=0)
    probsum = stats[Mt * E:2 * Mt * E].reshape(Mt, E).sum(axis=0)
    zsum = stats[2 * Mt * E:].sum()
    token_fractions = (counts / np.float32(T * TOPK)).astype(np.float32)
    mean_probs = (probsum / np.float32(T)).astype(np.float32)
    balance_loss = np.float32(E) * np.float32(np.sum(token_fractions * mean_probs))
    z_loss = np.float32(zsum / np.float32(T))

    # ---- host dispatch (the all-to-all): gather tokens per expert
    idx = [np.flatnonzero(comb[:, e]) for e in range(E)]
    max_n = max(len(i) for i in idx)
    C = max(TB, ((max_n + TB - 1) // TB) * TB)

    key = ("expert", C)
    if key not in _cache:
        _cache[key] = _build_expert(C)
    ncB = _cache[key]

    in_maps = []
    for e in range(E):
        n_e = len(idx[e])
        xg = np.zeros((C, H), dtype=np.float32)
        xg[:n_e] = x_flat[idx[e]]
        cve = np.zeros((1, C), dtype=np.float32)
        cve[0, :n_e] = comb[idx[e], e]
        in_maps.append({
            "xg": _round_f32r(np.ascontiguousarray(xg.T)),
            "wg": _round_f32r(w_gate[e]),
            "wu": _round_f32r(w_up[e]),
            "wd": _round_f32r(w_down[e]),
            "cv": cve,
        })
    resB = run_bass_kernel_spmd(ncB, in_maps, list(range(NCORES)), trace=trace)
    if trace:
        LAST_EXEC_NS["expert"] = resB.exec_time_ns

    # ---- host unshard: weighted outputs scatter-add back to token order
    out_flat = np.zeros((T, H), dtype=np.float32)
    for e in range(E):
        n_e = len(idx[e])
        if n_e:
            out_flat[idx[e]] += resB.results[e]["yT"][:, :n_e].T

    return out_flat.reshape(B, S, H), balance_loss, z_loss


# revision 9
# speedup vs baseline: 1.0392x; 1.0392x over previous
"""MoE (top-2 of 8 experts, SwiGLU FFN) on 8 trn2 NeuronCores.

Strategy:
  Launch A (data-parallel router): each core computes fp32 logits for T/8
    tokens, softmax, top-2 combine weights (exact renormalization), and
    partial aux-loss sums (counts, prob column-sums, sum lse^2).
  Host: builds per-expert token index lists from the device combine weights
    (the all-to-all dispatch), pads to a common capacity C.
  Launch B (expert-parallel FFN): core e holds expert e's weights and its
    gathered tokens (activations kept token-in-free-dim / transposed so all
    three matmuls use natural weight layouts with zero on-device transposes).
    Matmuls run in float32r (TF32-like, 11-bit mantissa -> exact products,
    fp32 PSUM accumulate) at 4x the fp32 PE rate. The per-token combine
    weight is applied on-device to the mm3 output.
  Host: scatter-adds per-expert outputs back to [T, H] (the unshard step)
    and combines aux partials into the two scalar losses.
"""

import sys
import types

sys.path.insert(0, "/opt/trn_rl_repo")

import numpy as np

import concourse.bass as bass  # noqa: F401  (bass must import before bacc)
import concourse.mybir as mybir
import concourse.tile as tile
from concourse import bacc
from concourse.bass_utils import run_bass_kernel_spmd

H, E, TOPK, F = 1024, 8, 2, 4096
NCORES = 8
TB = 768  # token block for the expert kernel (hT resident per block)
P = 128

TRACE = False  # test harness flips this to collect NTFF exec times
LAST_EXEC_NS = {}

FP32 = mybir.dt.float32
FP32R = mybir.dt.float32r

_cache = {}


def _install_trace_hook():
    try:
        from trn_agent_boot.trn_boot import _ntff_profile_via_ctypes

        hook = _ntff_profile_via_ctypes("/opt/axon/libaxon_pjrt.so")
        m = types.ModuleType("antenv.axon_hooks")
        m.get_axon_ntff_profile_hook = lambda: hook
        sys.modules["antenv.axon_hooks"] = m
        return True
    except Exception:
        return False


def _round_f32r(a):
    """Round fp32 -> fp32r (11-bit mantissa) the way the HW/static_cast does."""
    v = np.ascontiguousarray(a, dtype=np.float32).view(np.uint32).astype(np.uint64)
    v = (v + 0x800) & 0xFFFFF000
    return v.astype(np.uint32).view(np.float32)


def _build_router(Td):
    """SPMD router kernel: Td tokens per core, batched over all Mt m-tiles.

    Inputs:  xT [H, Td] fp32 (tokens of this core, transposed), gw [H, E] fp32
    Outputs: comb [Td, E] fp32 (renormalized top-2 combine weights, 0 elsewhere)
             stats [1, 2*Mt*E + Mt] fp32 =
               [mask colsums per (m,e) | prob colsums per (m,e) | lse^2 sums per m]
    No max-subtraction in the softmax: |logits| <~ 10 here, exp() is safe and
    probs/lse match the max-subtracted form to fp rounding.
    """
    KT = H // P
    Mt = Td // P
    assert Mt * E <= 512  # logits for all m-tiles live in one PSUM bank
    nc = bacc.Bacc("TRN2", target_bir_lowering=False, debug=False, num_devices=NCORES)
    xT = nc.dram_tensor("xT", [H, Td], FP32, kind="ExternalInput")
    gw = nc.dram_tensor("gw", [H, E], FP32, kind="ExternalInput")
    comb_o = nc.dram_tensor("comb", [Td, E], FP32, kind="ExternalOutput")
    NST = 2 * Mt * E + Mt
    stats_o = nc.dram_tensor("stats", [1, NST], FP32, kind="ExternalOutput")

    AF = mybir.ActivationFunctionType
    ALU = mybir.AluOpType
    from concourse.masks import make_identity
    with tile.TileContext(nc) as tc:
        with (
            tc.tile_pool(name="const", bufs=1) as const,
            tc.tile_pool(name="xp", bufs=1) as xp,
            tc.tile_pool(name="sp", bufs=1) as sp,
            tc.tile_pool(name="pp", bufs=1, space="PSUM") as pp,
            tc.tile_pool(name="ppt", bufs=2, space="PSUM") as ppt,
            tc.tile_pool(name="spp", bufs=1, space="PSUM") as spp,
        ):
            gsb = xp.tile([P, KT, E], FP32)
            nc.sync.dma_start(gsb[:], gw.rearrange("(k p) e -> p k e", p=P))
            xsb = xp.tile([P, KT, Td], FP32)
            for k in range(KT):
                nc.sync.dma_start(xsb[:, k, :], xT[k * P:(k + 1) * P, :])
            ones = const.tile([P, 1], FP32)
            nc.vector.memset(ones[:], 1.0)
            ident = const.tile([P, P], FP32)
            make_identity(nc, ident[:])

            # Compute logits TRANSPOSED [E, Td] with the tiny gate as the
            # stationary operand (8-column weight loads), k-inner so matmuls
            # chase the per-k xT DMAs; then PE-transpose m-tiles back into a
            # [128, Mt, E] PSUM bank.  (fp32 throughout: routing decisions
            # need full precision.)
            ltsb = sp.tile([E, Td], FP32, tag="ltsb")
            for c in range(Td // 512):
                pst = ppt.tile([E, 512], FP32, tag="pst")
                for k in range(KT):
                    nc.tensor.matmul(
                        pst[:],
                        gsb[:, k, :],
                        xsb[:, k, c * 512:(c + 1) * 512],
                        start=(k == 0),
                        stop=(k == KT - 1),
                    )
                nc.vector.tensor_copy(ltsb[:, c * 512:(c + 1) * 512], pst[:])
            # NOTE: accumulation groups must be contiguous in issue order —
            # interleaving groups within one bank computes garbage.
            ps = pp.tile([P, Mt, E], FP32)
            for m in range(Mt):
                nc.tensor.transpose(
                    ps[:, m, :], ltsb[:, m * P:(m + 1) * P], ident[0:E, 0:E]
                )

            def b3(t):  # [P, Mt] -> broadcast [P, Mt, E] access pattern
                return t[:].to_broadcast((P, Mt, E))

            ex = sp.tile([P, Mt, E], FP32, tag="ex")
            nc.scalar.activation(ex[:], ps[:], AF.Exp)
            sumexp = sp.tile([P, Mt], FP32, tag="sumexp")
            nc.vector.reduce_sum(sumexp[:], ex[:], axis=mybir.AxisListType.X)
            rs = sp.tile([P, Mt], FP32, tag="rs")
            nc.vector.reciprocal(rs[:], sumexp[:])
            probs = sp.tile([P, Mt, E], FP32, tag="probs")
            nc.vector.tensor_tensor(probs[:], ex[:], b3(rs), op=ALU.mult)

            top1 = sp.tile([P, Mt], FP32, tag="top1")
            nc.vector.reduce_max(top1[:], probs[:], axis=mybir.AxisListType.X)
            m1 = sp.tile([P, Mt, E], FP32, tag="m1")
            nc.vector.tensor_tensor(m1[:], probs[:], b3(top1), op=ALU.is_ge)
            pm = sp.tile([P, Mt, E], FP32, tag="pm")
            nc.vector.tensor_mul(pm[:], probs[:], m1[:])
            nc.vector.tensor_sub(pm[:], probs[:], pm[:])
            top2 = sp.tile([P, Mt], FP32, tag="top2")
            nc.vector.reduce_max(top2[:], pm[:], axis=mybir.AxisListType.X)
            mask = sp.tile([P, Mt, E], FP32, tag="mask")
            nc.vector.tensor_tensor(mask[:], pm[:], b3(top2), op=ALU.is_ge)
            nc.vector.tensor_add(mask[:], mask[:], m1[:])
            cu = sp.tile([P, Mt, E], FP32, tag="cu")
            nc.vector.tensor_mul(cu[:], probs[:], mask[:])
            denom = sp.tile([P, Mt], FP32, tag="denom")
            nc.vector.tensor_add(denom[:], top1[:], top2[:])
            rd = sp.tile([P, Mt], FP32, tag="rd")
            nc.vector.reciprocal(rd[:], denom[:])
            comb = sp.tile([P, Mt, E], FP32, tag="comb")
            nc.vector.tensor_tensor(comb[:], cu[:], b3(rd), op=ALU.mult)
            nc.sync.dma_start(comb_o.rearrange("(m p) e -> p m e", p=P), comb[:])

            lse = sp.tile([P, Mt], FP32, tag="lse")
            nc.scalar.activation(lse[:], sumexp[:], AF.Ln)
            z2 = sp.tile([P, Mt], FP32, tag="z2")
            nc.vector.tensor_mul(z2[:], lse[:], lse[:])

            # column-sums over the 128 token partitions via ones-matmul
            stats_ps = spp.tile([1, NST], FP32)
            nc.tensor.matmul(stats_ps[:, 0:Mt * E], ones[:], mask[:])
            nc.tensor.matmul(stats_ps[:, Mt * E:2 * Mt * E], ones[:], probs[:])
            nc.tensor.matmul(stats_ps[:, 2 * Mt * E:NST], ones[:], z2[:])
            stats_sb = sp.tile([1, NST], FP32, tag="stats_sb")
            nc.vector.tensor_copy(stats_sb[:], stats_ps[:])
            nc.sync.dma_start(stats_o[:], stats_sb[:])
    nc.compile()
    return nc, Mt


def _build_expert(C):
    """SPMD expert kernel: capacity C tokens per core (C multiple of TB).

    Inputs (all f32r except cv):
      xg [H, C]   gathered tokens, transposed
      wg, wu [H, F], wd [F, H]   expert weights, natural layouts
      cv [1, C] fp32             per-token combine weight (0 for padding)
    Output: yT [H, C] fp32 = comb * (silu(xg.T@wg) * (xg.T@wu)) @ wd, transposed
    """
    KT = H // P          # 8  k-tiles over H
    FT = F // P          # 32 f-tiles over F
    MT = H // P          # 8  m-tiles over H (mm3 out)
    NT = C // TB
    chunks = []
    c0 = 0
    while c0 < TB:
        cn = min(512, TB - c0)
        chunks.append((c0, cn))
        c0 += cn
    AF = mybir.ActivationFunctionType

    nc = bacc.Bacc("TRN2", target_bir_lowering=False, debug=False, num_devices=NCORES)
    xg = nc.dram_tensor("xg", [H, C], FP32R, kind="ExternalInput")
    wgd = nc.dram_tensor("wg", [H, F], FP32R, kind="ExternalInput")
    wud = nc.dram_tensor("wu", [H, F], FP32R, kind="ExternalInput")
    wdd = nc.dram_tensor("wd", [F, H], FP32R, kind="ExternalInput")
    cv = nc.dram_tensor("cv", [1, C], FP32, kind="ExternalInput")
    yT = nc.dram_tensor("yT", [H, C], FP32, kind="ExternalOutput")

    with tile.TileContext(nc) as tc:
        with (
            tc.tile_pool(name="cpool", bufs=1) as cpool,
            tc.tile_pool(name="xpool", bufs=1) as xpool,
            tc.tile_pool(name="hpool", bufs=1) as hpool,
            tc.tile_pool(name="wpool", bufs=3) as wpool,
            tc.tile_pool(name="wdpool", bufs=2) as wdpool,
            tc.tile_pool(name="h1pool", bufs=3) as h1pool,
            tc.tile_pool(name="ypool", bufs=3) as ypool,
            tc.tile_pool(name="ppa", bufs=2, space="PSUM") as ppa,
            tc.tile_pool(name="ppb", bufs=3, space="PSUM") as ppb,
        ):
            comb_sb = cpool.tile([P, C], FP32)
            nc.sync.dma_start(comb_sb[:], cv[0:1, :].to_broadcast((P, C)))

            def load_w(f):
                wgsb = wpool.tile([P, KT, P], FP32R, tag="wgsb")
                nc.sync.dma_start(
                    wgsb[:],
                    wgd[:, f * P:(f + 1) * P].rearrange("(k p) m -> p k m", p=P),
                )
                wusb = wpool.tile([P, KT, P], FP32R, tag="wusb")
                nc.sync.dma_start(
                    wusb[:],
                    wud[:, f * P:(f + 1) * P].rearrange("(k p) m -> p k m", p=P),
                )
                return wgsb, wusb

            for tb in range(NT):
                tok0 = tb * TB
                # f=0 weights issued BEFORE the xsb bulk so the first matmul
                # gates on ~3MB of DMA, not the full 8MB
                w_pre = load_w(0) if tb == 0 else None
                xsb = xpool.tile([P, KT, TB], FP32R, tag="xsb")
                for (c0, cn) in chunks:
                    for k in range(KT):
                        nc.sync.dma_start(
                            xsb[:, k, c0:c0 + cn],
                            xg[k * P:(k + 1) * P, tok0 + c0:tok0 + c0 + cn],
                        )
                hT = hpool.tile([P, FT, TB], FP32R, tag="hT")

                # ---- phase A: hT = f32r(silu(x@wg) * (x@wu)), F on partitions
                for f in range(FT):
                    wgsb, wusb = w_pre if (tb == 0 and f == 0) else load_w(f)
                    for (c0, cn) in chunks:
                        ps1 = ppa.tile([P, 512], FP32, tag="ps1")
                        for k in range(KT):
                            nc.tensor.matmul(
                                ps1[:, :cn],
                                wgsb[:, k, :],
                                xsb[:, k, c0:c0 + cn],
                                start=(k == 0),
                                stop=(k == KT - 1),
                            )
                        h1 = h1pool.tile([P, 512], FP32, tag="h1")
                        nc.scalar.activation(h1[:, :cn], ps1[:, :cn], AF.Silu)
                        ps2 = ppa.tile([P, 512], FP32, tag="ps2")
                        for k in range(KT):
                            nc.tensor.matmul(
                                ps2[:, :cn],
                                wusb[:, k, :],
                                xsb[:, k, c0:c0 + cn],
                                start=(k == 0),
                                stop=(k == KT - 1),
                            )
                        nc.vector.tensor_mul(
                            hT[:, f, c0:c0 + cn], h1[:, :cn], ps2[:, :cn]
                        )

                # ---- phase B: yT = comb * (hT.T @ wd).T, H on partitions
                for m in range(MT):
                    wdsb = wdpool.tile([P, FT, P], FP32R, tag="wdsb")
                    nc.sync.dma_start(
                        wdsb[:],
                        wdd[:, m * P:(m + 1) * P].rearrange("(k p) h -> p k h", p=P),
                    )
                    for (c0, cn) in chunks:
                        ps = ppb.tile([P, 512], FP32, tag="ps3")
                        for k in range(FT):
                            nc.tensor.matmul(
                                ps[:, :cn],
                                wdsb[:, k, :],
                                hT[:, k, c0:c0 + cn],
                                start=(k == 0),
                                stop=(k == FT - 1),
                            )
                        yo = ypool.tile([P, 512], FP32, tag="yo")
                        nc.vector.tensor_mul(
                            yo[:, :cn], ps[:, :cn],
                            comb_sb[:, tok0 + c0:tok0 + c0 + cn],
                        )
                        nc.sync.dma_start(
                            yT[m * P:(m + 1) * P, tok0 + c0:tok0 + c0 + cn],
                            yo[:, :cn],
                        )
    nc.compile()
    return nc


def kernel(x, gate_w, w_gate, w_up, w_down):
    global LAST_EXEC_NS
    LAST_EXEC_NS = {}
    trace = TRACE and _install_trace_hook()

    x = np.ascontiguousarray(np.asarray(x, dtype=np.float32))
    gate_w = np.ascontiguousarray(np.asarray(gate_w, dtype=np.float32))
    w_gate = np.ascontiguousarray(np.asarray(w_gate, dtype=np.float32))
    w_up = np.ascontiguousarray(np.asarray(w_up, dtype=np.float32))
    w_down = np.ascontiguousarray(np.asarray(w_down, dtype=np.float32))

    B, S, _ = x.shape
    T = B * S
    assert T % NCORES == 0
    Td = T // NCORES
    x_flat = x.reshape(T, H)

    # ---- launch A: router
    key = ("router", Td)
    if key not in _cache:
        _cache[key] = _build_router(Td)
    ncA, Mt = _cache[key]
    in_maps = [
        {"xT": np.ascontiguousarray(x_flat[d * Td:(d + 1) * Td].T), "gw": gate_w}
        for d in range(NCORES)
    ]
    resA = run_bass_kernel_spmd(ncA, in_maps, list(range(NCORES)), trace=trace)
    if trace:
        LAST_EXEC_NS["router"] = resA.exec_time_ns
    comb = np.concatenate([r["comb"] for r in resA.results], axis=0)  # [T, E]
    stats = np.sum([r["stats"][0] for r in resA.results], axis=0)
    counts = stats[0:Mt * E].reshape(Mt, E).sum(axis=0)
    probsum = stats[Mt * E:2 * Mt * E].reshape(Mt, E).sum(axis=0)
    zsum = stats[2 * Mt * E:].sum()
    token_fractions = (counts / np.float32(T * TOPK)).astype(np.float32)
    mean_probs = (probsum / np.float32(T)).astype(np.float32)
    balance_loss = np.float32(E) * np.float32(np.sum(token_fractions * mean_probs))
    z_loss = np.float32(zsum / np.float32(T))

    # ---- host dispatch (the all-to-all): gather tokens per expert
    idx = [np.flatnonzero(comb[:, e]) for e in range(E)]
    max_n = max(len(i) for i in idx)
    C = max(TB, ((max_n + TB - 1) // TB) * TB)

    key = ("expert", C)
    if key not in _cache:
        _cache[key] = _build_expert(C)
    ncB = _cache[key]

    in_maps = []
    for e in range(E):
        n_e = len(idx[e])
        xg = np.zeros((C, H), dtype=np.float32)
        xg[:n_e] = x_flat[idx[e]]
        cve = np.zeros((1, C), dtype=np.float32)
        cve[0, :n_e] = comb[idx[e], e]
        in_maps.append({
            "xg": _round_f32r(np.ascontiguousarray(xg.T)),
            "wg": _round_f32r(w_gate[e]),
            "wu": _round_f32r(w_up[e]),
            "wd": _round_f32r(w_down[e]),
            "cv": cve,
        })
    resB = run_bass_kernel_spmd(ncB, in_maps, list(range(NCORES)), trace=trace)
    if trace:
        LAST_EXEC_NS["expert"] = resB.exec_time_ns

    # ---- host unshard: weighted outputs scatter-add back to token order
    out_flat = np.zeros((T, H), dtype=np.float32)
    for e in range(E):
        n_e = len(idx[e])
        if n_e:
            out_flat[idx[e]] += resB.results[e]["yT"][:, :n_e].T

    return out_flat.reshape(B, S, H), balance_loss, z_loss
